# revision 26
# baseline (speedup 1.0000x reference)
"""Bidirectional Mamba block on 8 Trainium2 NeuronCores.

Sharding: core c in 0..7 handles (batch b = c % 4, direction d = c // 4).
The two directions of one batch are independent branches until the final
out_proj-sum + residual + RMSNorm, which a second tiny SPMD kernel does
(8 cores = 4 batches x 2 sequence halves).

Stage A (per core): LayerNorm -> in_proj -> causal dwconv+SiLU -> x_proj
-> dt_proj+softplus -> selective scan (tensor_tensor_scan along L, one
(e-tile, n) plane at a time) -> silu(z) gate -> out_proj partial.
Host only does slicing / transposes / flips (layout, no math).
"""

import sys
import numpy as np

sys.path.insert(0, "/opt/trn_rl_repo")

B, L, D, E, N, KC, R = 4, 2048, 512, 1024, 16, 4, 32
EPS = 1e-5
ET = E // 128       # 8 e-tiles
DT = D // 128       # 4 d-tiles
TL = 1024           # L chunk size
POOL16 = 16         # of 16 scan-iters, how many offload gg/y+= to GpSimd
NL = L // TL        # chunks
NSUB = TL // 512    # 512-wide matmul subchunks per chunk

_cache = {}


def _build_stage_a(reps=1):
    import concourse.tile as tile
    from concourse import bacc, mybir
    from concourse.alu_op_type import AluOpType as op
    from contextlib import ExitStack

    dt = mybir.dt
    f32, f16 = dt.float32, dt.float16
    AF = mybir.ActivationFunctionType

    nc = bacc.Bacc("TRN2", target_bir_lowering=False, debug=False, num_devices=8)

    # ---- DRAM I/O (per-core values supplied via in_maps) ----
    hsT = nc.dram_tensor("hsT", [D, L], f16, kind="ExternalInput").ap()
    w_inT = nc.dram_tensor("w_inT", [D, 2 * E], f16, kind="ExternalInput").ap()
    out_wT = nc.dram_tensor("out_wT", [E, D], f16, kind="ExternalInput").ap()
    xp_wT = nc.dram_tensor("xp_wT", [E, R + 2 * N], f16, kind="ExternalInput").ap()
    dtp_wT = nc.dram_tensor("dtp_wT", [R, E], f16, kind="ExternalInput").ap()
    # packed per-partition columns: [conv_w(4) per tile | conv_b | dt_b | D | norm cols]
    convw = nc.dram_tensor("convw", [128, ET * KC], f32, kind="ExternalInput").ap()
    convb = nc.dram_tensor("convb", [128, ET], f32, kind="ExternalInput").ap()
    dtb = nc.dram_tensor("dtb", [128, ET], f32, kind="ExternalInput").ap()
    dvec = nc.dram_tensor("dvec", [128, ET], f32, kind="ExternalInput").ap()
    alog = nc.dram_tensor("alog", [128, ET * N], f32, kind="ExternalInput").ap()
    nw = nc.dram_tensor("nw", [128, DT], f32, kind="ExternalInput").ap()
    nb = nc.dram_tensor("nb", [128, DT], f32, kind="ExternalInput").ap()
    y_part = nc.dram_tensor("y_part", [D, L], f32, kind="ExternalOutput").ap()
    bcd = nc.dram_tensor("bcd", [2 * N, L], f16).ap()  # B/C rows bounce buffer
    zdram = nc.dram_tensor("zdram", [E, L], f16).ap()  # z half spill

    with tile.TileContext(nc) as tc:
        with ExitStack() as ctx:
            P = 128

            def pool(name, bufs):
                return ctx.enter_context(tc.tile_pool(name=name, bufs=bufs))

            pers = pool("pers", 1)
            ps_pool = ctx.enter_context(tc.tile_pool(name="ps", bufs=3, space="PSUM"))
            ps_stat = ctx.enter_context(tc.tile_pool(name="psst", bufs=1, space="PSUM"))
            ps_small = ctx.enter_context(tc.tile_pool(name="pssm", bufs=2, space="PSUM"))

            # ---- persistent weight tiles ----
            w_in = [pers.tile([P, 2 * E], f16, tag=f"win{k}") for k in range(DT)]
            for k in range(DT):
                nc.sync.dma_start(w_in[k][:], w_inT[128 * k:128 * (k + 1), :])
            out_w = [pers.tile([P, D], f16, tag=f"ow{i}") for i in range(ET)]
            for i in range(ET):
                nc.sync.dma_start(out_w[i][:], out_wT[128 * i:128 * (i + 1), :])
            xp_w = [pers.tile([P, R + 2 * N], f16, tag=f"xpw{i}") for i in range(ET)]
            for i in range(ET):
                nc.sync.dma_start(xp_w[i][:], xp_wT[128 * i:128 * (i + 1), :])
            dtp_w = pers.tile([R, E], f16, tag="dtpw")
            nc.sync.dma_start(dtp_w[:], dtp_wT[:])
            cw = pers.tile([P, ET * KC], f32, tag="cw")
            nc.sync.dma_start(cw[:], convw[:])
            cb = pers.tile([P, ET], f32, tag="cb")
            nc.sync.dma_start(cb[:], convb[:])
            dtbt = pers.tile([P, ET], f32, tag="dtb")
            nc.sync.dma_start(dtbt[:], dtb[:])
            dvt = pers.tile([P, ET], f32, tag="dv")
            nc.sync.dma_start(dvt[:], dvec[:])
            alg = pers.tile([P, ET * N], f32, tag="alog")
            nc.sync.dma_start(alg[:], alog[:])
            nwt = pers.tile([P, DT], f32, tag="nw")
            nc.sync.dma_start(nwt[:], nw[:])
            nbt = pers.tile([P, DT], f32, tag="nb")
            nc.sync.dma_start(nbt[:], nb[:])

            ones = pers.tile([P, 1], f16, tag="ones")
            nc.vector.memset(ones[:], 1.0)

            # A = -exp(A_log)  (128, ET*N) f32
            At = pers.tile([P, ET * N], f32, tag="A")
            nc.scalar.activation(At[:], alg[:], AF.Exp)
            nc.vector.tensor_scalar_mul(At[:], At[:], -1.0)

            # scan carry state h[:, (i,n)] and conv tails
            carry = [pers.tile([P, N], f16, tag=f"carry{i}") for i in range(ET)]
            xtail = [pers.tile([P, 3], f16, tag=f"xtail{i}") for i in range(ET)]
            for i in range(ET):
                nc.vector.memset(xtail[i][:], 0.0)

            # ---- streaming pools ----
            hst_p = pool("hst", 1)     # hsT chunk tiles (f16)
            sq_p = pool("sq", 1)       # squared tiles (f16)
            srow_p = pool("srow", 1)   # stat rows (1, TL) f32
            rep_p = pool("rep", 1)     # broadcast stat rows (128, TL) f32
            hn_p = pool("hn", 1)       # normalized hs (f16), DT tags
            xpad_p = pool("xpad", 2)   # conv input [3 | TL] f16, cycled
            xc_p = pool("xc", 1)       # conv output f16, ET tags
            z_p = pool("z", 2)         # z spill bounce (small)
            xdbl_p = pool("xdbl", 2)   # (64, TL) f16
            dl_p = pool("dl", 1)       # delta f16, ET tags
            du_p = pool("du", 1)       # delta*u f16, ET tags
            y_p = pool("y", 1)         # y accum f32, ET tags
            bc_p = pool("bc", 3)       # B/C broadcast planes f16
            tr_p = pool("tr", 2)       # scan transients f16
            ov_p = pool("ov", 1)       # out_proj result f32
            tmp_p = pool("tmp", 1)     # misc small

            import itertools
            for rep, c in itertools.product(range(reps), range(NL)):
                lo = c * TL
                # ---------- LayerNorm (over D, layout (D,L)) ----------
                hst = []
                for k in range(DT):
                    t = hst_p.tile([P, TL], f16, tag=f"hst{k}")
                    nc.sync.dma_start(t[:], hsT[128 * k:128 * (k + 1), lo:lo + TL])
                    hst.append(t)
                mu_ps = ps_stat.tile([1, TL], f32, tag="mups")
                sq_ps = ps_stat.tile([1, TL], f32, tag="sqps")
                for k in range(DT):
                    sqt = sq_p.tile([P, TL], f16, tag="sq")
                    nc.scalar.square(sqt[:], hst[k][:])
                    for s in range(NSUB):
                        sl = slice(512 * s, 512 * (s + 1))
                        nc.tensor.matmul(mu_ps[:, sl], ones[:], hst[k][:, sl],
                                         start=(k == 0), stop=(k == DT - 1))
                        nc.tensor.matmul(sq_ps[:, sl], ones[:], sqt[:, sl],
                                         start=(k == 0), stop=(k == DT - 1))
                mu = srow_p.tile([1, TL], f32, tag="mu")
                nc.scalar.activation(mu[:], mu_ps[:], AF.Copy, scale=1.0 / D)
                msq = srow_p.tile([1, TL], f32, tag="msq")
                nc.scalar.activation(msq[:], sq_ps[:], AF.Copy, scale=1.0 / D)
                var = srow_p.tile([1, TL], f32, tag="var")
                nc.vector.tensor_tensor(var[:], mu[:], mu[:], op=op.mult)
                nc.vector.tensor_sub(var[:], msq[:], var[:])
                lnv = srow_p.tile([1, TL], f32, tag="lnv")
                nc.scalar.activation(lnv[:], var[:], AF.Ln, bias=epst[0:1, :])
                rs = srow_p.tile([1, TL], f32, tag="rs")
                nc.scalar.activation(rs[:], lnv[:], AF.Exp, scale=-0.5)
                murep = rep_p.tile([P, TL], f32, tag="murep")
                nc.gpsimd.partition_broadcast(murep[:], mu[:])
                rsrep = rep_p.tile([P, TL], f32, tag="rsrep")
                nc.gpsimd.partition_broadcast(rsrep[:], rs[:])
                hn = []
                for k in range(DT):
                    rsw = tmp_p.tile([P, TL], f32, tag="rsw")
                    nc.vector.tensor_scalar_mul(rsw[:], rsrep[:], nwt[:, k:k + 1])
                    bias2 = tmp_p.tile([P, TL], f32, tag="bias2")
                    nc.vector.tensor_tensor(bias2[:], murep[:], rsw[:], op=op.mult)
                    nc.vector.tensor_scalar(bias2[:], bias2[:], -1.0, nbt[:, k:k + 1],
                                            op0=op.mult, op1=op.add)
                    ht = hn_p.tile([P, TL], f16, tag=f"hn{k}")
                    nc.vector.tensor_tensor(ht[:], hst[k][:], rsw[:], op=op.mult)
                    nc.vector.tensor_add(ht[:], ht[:], bias2[:])
                    hn.append(ht)

                # ---------- in_proj: xz[m, t] ----------
                xpads, zs = [], []
                for m in range(2 * ET):
                    ps = ps_pool.tile([P, TL], f32, tag="mm")
                    for k in range(DT):
                        for s in range(NSUB):
                            sl = slice(512 * s, 512 * (s + 1))
                            nc.tensor.matmul(ps[:, sl],
                                             w_in[k][:, 128 * m:128 * (m + 1)],
                                             hn[k][:, sl],
                                             start=(k == 0), stop=(k == DT - 1))
                    if m < ET:
                        xp = xpad_p.tile([P, TL + 3], f16, tag="xpad")
                        nc.scalar.copy(xp[:, 0:3], xtail[m][:])
                        nc.scalar.copy(xp[:, 3:TL + 3], ps[:])
                        xpads.append(xp)
                    else:
                        zt = z_p.tile([P, TL], f16, tag=f"z{m - ET}")
                        nc.scalar.copy(zt[:], ps[:])
                        zs.append(zt)

                # ---------- causal dwconv + SiLU ----------
                xcs = []
                for i in range(ET):
                    xp = xpads[i]
                    acc = tmp_p.tile([P, TL], f16, tag="cacc")
                    nc.vector.tensor_scalar_mul(acc[:], xp[:, 0:TL], cw[:, KC * i:KC * i + 1])
                    for k in range(1, KC):
                        nc.vector.scalar_tensor_tensor(
                            acc[:], xp[:, k:TL + k], cw[:, KC * i + k:KC * i + k + 1],
                            acc[:], op0=op.mult, op1=op.add)
                    # save tail for next chunk, then silu(acc + conv_b)
                    nc.scalar.copy(xtail[i][:], xp[:, TL:TL + 3])
                    xct = xc_p.tile([P, TL], f16, tag=f"xc{i}")
                    nc.scalar.activation(xct[:], acc[:], AF.Silu, bias=cb[:, i:i + 1])
                    xcs.append(xct)

                # ---------- x_proj ----------
                xdbl = xdbl_p.tile([R + 2 * N, TL], f16, tag="xdbl")
                ps = ps_small.tile([R + 2 * N, TL], f32, tag="xdblps")
                for i in range(ET):
                    for s in range(NSUB):
                        sl = slice(512 * s, 512 * (s + 1))
                        nc.tensor.matmul(ps[:, sl], xp_w[i][:], xcs[i][:, sl],
                                         start=(i == 0), stop=(i == ET - 1))
                nc.scalar.copy(xdbl[:], ps[0:R, :])
                bcs = xdbl_p.tile([2 * N, TL], f16, tag="bcs", name="bcs", bufs=1)
                nc.scalar.copy(bcs[:], ps[R:R + 2 * N, :])
                nc.sync.dma_start(bcd[:, lo:lo + TL], bcs[:])

                # ---------- dt_proj + softplus ----------
                dls, dus, ys = [], [], []
                for i in range(ET):
                    ps = ps_pool.tile([P, TL], f32, tag="mm")
                    for s in range(NSUB):
                        sl = slice(512 * s, 512 * (s + 1))
                        nc.tensor.matmul(ps[:, sl], dtp_w[:, 128 * i:128 * (i + 1)],
                                         xdbl[0:R, sl], start=True, stop=True)
                    esp = tmp_p.tile([P, TL], f32, tag="esp")
                    nc.scalar.activation(esp[:], ps[:], AF.Exp, bias=dtbt[:, i:i + 1])
                    dl = dl_p.tile([P, TL], f16, tag=f"dl{i}")
                    nc.scalar.activation(dl[:], esp[:], AF.Ln, bias=1.0)
                    dls.append(dl)
                    du = du_p.tile([P, TL], f16, tag=f"du{i}")
                    nc.vector.tensor_tensor(du[:], dl[:], xcs[i][:], op=op.mult)
                    dus.append(du)
                    yt = y_p.tile([P, TL], f32, tag=f"y{i}")
                    nc.vector.tensor_scalar_mul(yt[:], xcs[i][:], dvt[:, i:i + 1])
                    ys.append(yt)

                # ---------- selective scan ----------
                for n in range(N):
                    bp = bc_p.tile([P, TL], f16, tag="bp")
                    nc.gpsimd.partition_broadcast(bp[:], xdbl[R + n:R + n + 1, :])
                    cp = bc_p.tile([P, TL], f16, tag="cp")
                    nc.gpsimd.partition_broadcast(cp[:], xdbl[R + N + n:R + N + n + 1, :])
                    for i in range(ET):
                        da = tr_p.tile([P, TL], f16, tag="da")
                        nc.scalar.activation(da[:], dls[i][:], AF.Exp,
                                             scale=At[:, N * i + n:N * i + n + 1])
                        db = tr_p.tile([P, TL], f16, tag="db")
                        nc.vector.tensor_tensor(db[:], dus[i][:], bp[:], op=op.mult)
                        hh = tr_p.tile([P, TL], f16, tag="hh")
                        init = 0.0 if c == 0 else carry[i][:, n:n + 1]
                        nc.vector.tensor_tensor_scan(hh[:], da[:], db[:], init,
                                                     op0=op.mult, op1=op.add)
                        if c < NL - 1:
                            nc.scalar.copy(carry[i][:, n:n + 1], hh[:, TL - 1:TL])
                        gg = tr_p.tile([P, TL], f16, tag="gg")
                        nc.vector.tensor_tensor(gg[:], hh[:], cp[:], op=op.mult)
                        nc.vector.tensor_add(ys[i][:], ys[i][:], gg[:])

                # ---------- gate + out_proj ----------
                ygs = []
                for i in range(ET):
                    zst = tmp_p.tile([P, TL], f16, tag="zs")
                    nc.scalar.activation(zst[:], zs[i][:], AF.Silu)
                    yg = yg_p.tile([P, TL], f16, tag=f"yg{i}")
                    nc.vector.tensor_tensor(yg[:], ys[i][:], zst[:], op=op.mult)
                    ygs.append(yg)
                for m in range(DT):
                    ps = ps_pool.tile([P, TL], f32, tag="mm")
                    for i in range(ET):
                        for s in range(NSUB):
                            sl = slice(512 * s, 512 * (s + 1))
                            nc.tensor.matmul(ps[:, sl],
                                             out_w[i][:, 128 * m:128 * (m + 1)],
                                             ygs[i][:, sl],
                                             start=(i == 0), stop=(i == ET - 1))
                    ov = ov_p.tile([P, TL], f32, tag="ov")
                    nc.scalar.copy(ov[:], ps[:])
                    nc.sync.dma_start(y_part[128 * m:128 * (m + 1), lo:lo + TL], ov[:])

    nc.compile()
    return nc


def _build_stage_b(reps=1):
    import concourse.tile as tile
    from concourse import bacc, mybir
    from concourse.alu_op_type import AluOpType as op
    from contextlib import ExitStack

    dt = mybir.dt
    f32 = dt.float32
    AF = mybir.ActivationFunctionType
    LH = L // 2  # 1024 rows per core

    nc = bacc.Bacc("TRN2", target_bir_lowering=False, debug=False, num_devices=8)
    yf = nc.dram_tensor("yf", [LH, D], f32, kind="ExternalInput").ap()
    yr = nc.dram_tensor("yr", [LH, D], f32, kind="ExternalInput").ap()
    res = nc.dram_tensor("res", [LH, D], f32, kind="ExternalInput").ap()
    nfw = nc.dram_tensor("nfw", [1, D], f32, kind="ExternalInput").ap()
    out = nc.dram_tensor("out", [LH, D], f32, kind="ExternalOutput").ap()

    with tile.TileContext(nc) as tc:
        with ExitStack() as ctx:
            P = 128
            pers = ctx.enter_context(tc.tile_pool(name="pers", bufs=1))
            io_p = ctx.enter_context(tc.tile_pool(name="io", bufs=3))
            tmp_p = ctx.enter_context(tc.tile_pool(name="tmp", bufs=3))

            nfwt = pers.tile([1, D], f32, tag="nfw")
            nc.sync.dma_start(nfwt[:], nfw[:])
            nfr = pers.tile([P, D], f32, tag="nfr")
            nc.gpsimd.partition_broadcast(nfr[:], nfwt[:])

            import itertools
            for rep, t in itertools.product(range(reps), range(LH // P)):
                rows = slice(P * t, P * (t + 1))
                tf = io_p.tile([P, D], f32, tag="tf")
                nc.sync.dma_start(tf[:], yf[rows, :])
                tr = io_p.tile([P, D], f32, tag="tr")
                nc.sync.dma_start(tr[:], yr[rows, :])
                tres = io_p.tile([P, D], f32, tag="tres")
                nc.sync.dma_start(tres[:], res[rows, :])
                s = tmp_p.tile([P, D], f32, tag="s")
                nc.vector.tensor_add(s[:], tf[:], tr[:])
                nc.vector.tensor_add(s[:], s[:], tres[:])
                sq = tmp_p.tile([P, D], f32, tag="sq")
                ssum = tmp_p.tile([P, 1], f32, tag="ssum")
                nc.scalar.activation(sq[:], s[:], AF.Square, accum_out=ssum[:])
                lnm = tmp_p.tile([P, 1], f32, tag="lnm")
                nc.scalar.activation(lnm[:], ssum[:], AF.Ln, bias=epst[:], scale=1.0 / D)
                rinv = tmp_p.tile([P, 1], f32, tag="rinv")
                nc.scalar.activation(rinv[:], lnm[:], AF.Exp, scale=-0.5)
                o = tmp_p.tile([P, D], f32, tag="o")
                nc.vector.scalar_tensor_tensor(o[:], s[:], rinv[:], nfr[:],
                                               op0=op.mult, op1=op.mult)
                nc.sync.dma_start(out[rows, :], o[:])

    nc.compile()
    return nc


class _Runner:
    """Compile a Bass program once into a sharded PJRT callable for 8 cores."""

    def __init__(self, nc, n_cores=8):
        import jax
        import jax.numpy as jnp
        from jax.sharding import Mesh, PartitionSpec
        from jax.experimental.shard_map import shard_map
        from concourse import bass2jax, mybir

        bass2jax.install_neuronx_cc_hook()
        self.n_cores = n_cores
        in_names, out_names, out_avals, zero_outs = [], [], [], []
        partition_name = nc.partition_id_tensor.name if nc.partition_id_tensor else None
        for alloc in nc.m.functions[0].allocations:
            if not isinstance(alloc, mybir.MemoryLocationSet):
                continue
            name = alloc.memorylocations[0].name
            if alloc.kind == "ExternalInput":
                if name != partition_name:
                    in_names.append(name)
            elif alloc.kind == "ExternalOutput":
                shape = tuple(alloc.tensor_shape)
                dtype = mybir.dt.np(alloc.dtype)
                out_names.append(name)
                out_avals.append(jax.core.ShapedArray(shape, dtype))
                zero_outs.append(np.zeros((n_cores * shape[0],) + shape[1:], dtype))
        self.in_names, self.out_names, self.out_avals = in_names, out_names, out_avals
        n_params, n_outs = len(in_names), len(out_names)
        all_names = list(in_names) + list(out_names)
        if partition_name is not None:
            all_names.append(partition_name)

        def _body(*args):
            operands = list(args)
            if partition_name is not None:
                operands.append(bass2jax.partition_id_tensor())
            outs = bass2jax._bass_exec_p.bind(
                *operands,
                out_avals=tuple(out_avals),
                in_names=tuple(all_names),
                out_names=tuple(out_names),
                lowering_input_output_aliases=(),
                sim_require_finite=True,
                sim_require_nnan=True,
                nc=nc,
            )
            return tuple(outs)

        devices = jax.devices()[:n_cores]
        mesh = Mesh(np.asarray(devices), ("core",))
        in_specs = (PartitionSpec("core"),) * (n_params + n_outs)
        out_specs = (PartitionSpec("core"),) * n_outs
        self.fn = jax.jit(
            shard_map(_body, mesh=mesh, in_specs=in_specs,
                      out_specs=out_specs, check_rep=False),
            keep_unused=True)
        self.mesh = mesh
        self._zero_dev = [jax.device_put(z) for z in zero_outs]

    def prep(self, in_maps):
        import jax
        assert len(in_maps) == self.n_cores
        concat = [np.concatenate([np.asarray(m[n]) for m in in_maps], axis=0)
                  for n in self.in_names]
        return [jax.device_put(a) for a in concat] + self._zero_dev

    def run_dev(self, dev_args):
        return self.fn(*dev_args)

    def __call__(self, in_maps):
        import jax
        out_arrs = self.fn(*self.prep(in_maps))
        out_arrs = [np.asarray(a) for a in out_arrs]
        res = []
        for c in range(self.n_cores):
            d = {}
            for i, name in enumerate(self.out_names):
                shape = self.out_avals[i].shape
                d[name] = out_arrs[i].reshape((self.n_cores,) + shape)[c]
            res.append(d)
        return res


def _programs():
    if "a" not in _cache:
        _cache["a"] = _Runner(_build_stage_a())
        _cache["b"] = _Runner(_build_stage_b())
    return _cache["a"], _cache["b"]


def _pack_cols(v, width):
    # (E,)-like flat -> (128, ET*width) per-partition column blocks
    a = np.asarray(v, np.float32).reshape(ET, 128, width)
    return np.ascontiguousarray(a.transpose(1, 0, 2).reshape(128, ET * width))


def kernel(**inputs):
    run_a, run_b = _programs()
    f16 = np.float16
    hs = np.asarray(inputs["hidden_states"], np.float32)

    w_inT = np.ascontiguousarray(np.asarray(inputs["in_proj_w"], np.float32).T).astype(f16)
    out_wT = np.ascontiguousarray(np.asarray(inputs["out_proj_w"], np.float32).T).astype(f16)
    # norm_w/b are per-D; in (D,L) layout D is the partition dim -> column k = rows 128k..128k+127
    nw = np.ascontiguousarray(np.asarray(inputs["norm_w"], np.float32).reshape(DT, 128).T)
    nb = np.ascontiguousarray(np.asarray(inputs["norm_b"], np.float32).reshape(DT, 128).T)

    per_dir = {}
    for d, sfx in ((0, ""), (1, "_b")):
        per_dir[d] = dict(
            xp_wT=np.ascontiguousarray(np.asarray(inputs["x_proj_w" + sfx], np.float32).T).astype(f16),
            dtp_wT=np.ascontiguousarray(np.asarray(inputs["dt_proj_w" + sfx], np.float32).T).astype(f16),
            convw=_pack_cols(inputs["conv_w" + sfx], KC),
            convb=_pack_cols(inputs["conv_b" + sfx], 1),
            dtb=_pack_cols(inputs["dt_proj_b" + sfx], 1),
            alog=_pack_cols(inputs["A_log" if d == 0 else "A_b_log"], N),
            dvec=_pack_cols(inputs["D_fwd" if d == 0 else "D_bwd"], 1),
        )

    in_maps = []
    for c in range(8):
        b, d = c % 4, c // 4
        h = hs[b] if d == 0 else hs[b, ::-1]
        in_maps.append(dict(
            hsT=np.ascontiguousarray(h.T).astype(f16),
            w_inT=w_inT, out_wT=out_wT, nw=nw, nb=nb,
            **per_dir[d],
        ))
    _cache["last_in_maps_a"] = in_maps
    res_a = run_a(in_maps)

    LH = L // 2
    nfw = np.asarray(inputs["normf_w"], np.float32).reshape(1, D)
    in_maps_b = []
    for c in range(8):
        b, half = c % 4, c // 4
        rows = slice(half * LH, (half + 1) * LH)
        yfT = res_a[b]["y_part"].T            # (L, D)
        yrT = res_a[b + 4]["y_part"][:, ::-1].T
        in_maps_b.append(dict(
            yf=np.ascontiguousarray(yfT[rows]),
            yr=np.ascontiguousarray(yrT[rows]),
            res=np.ascontiguousarray(hs[b, rows]),
            nfw=nfw,
        ))
    _cache["last_in_maps_b"] = in_maps_b
    res_b = run_b(in_maps_b)

    out = np.empty((B, L, D), np.float32)
    for c in range(8):
        b, half = c % 4, c // 4
        out[b, half * LH:(half + 1) * LH] = res_b[c]["out"]
    return out


# revision 30
# speedup vs baseline: 1.1379x; 1.1379x over previous
"""Bidirectional Mamba block on 8 Trainium2 NeuronCores.

Sharding: core c in 0..7 handles (batch b = c % 4, direction d = c // 4).
The two directions of one batch are independent branches until the final
out_proj-sum + residual + RMSNorm, which a second tiny SPMD kernel does
(8 cores = 4 batches x 2 sequence halves).

Stage A (per core): LayerNorm -> in_proj -> causal dwconv+SiLU -> x_proj
-> dt_proj+softplus -> selective scan (tensor_tensor_scan along L, one
(e-tile, n) plane at a time) -> silu(z) gate -> out_proj partial.
Host only does slicing / transposes / flips (layout, no math).
"""

import sys
import numpy as np

sys.path.insert(0, "/opt/trn_rl_repo")

B, L, D, E, N, KC, R = 4, 2048, 512, 1024, 16, 4, 32
EPS = 1e-5
ET = E // 128       # 8 e-tiles
DT = D // 128       # 4 d-tiles
TL = 1024           # L chunk size
POOL16 = 16         # of 16 scan-iters, how many offload gg/y+= to GpSimd
NL = L // TL        # chunks
NSUB = TL // 512    # 512-wide matmul subchunks per chunk

_cache = {}


def _build_stage_a(reps=1):
    import concourse.tile as tile
    from concourse import bacc, mybir
    from concourse.alu_op_type import AluOpType as op
    from contextlib import ExitStack

    dt = mybir.dt
    f32, f16 = dt.float32, dt.float16
    AF = mybir.ActivationFunctionType

    nc = bacc.Bacc("TRN2", target_bir_lowering=False, debug=False, num_devices=8)

    # ---- DRAM I/O (per-core values supplied via in_maps) ----
    hsT = nc.dram_tensor("hsT", [D, L], f16, kind="ExternalInput").ap()
    w_inT = nc.dram_tensor("w_inT", [D, 2 * E], f16, kind="ExternalInput").ap()
    out_wT = nc.dram_tensor("out_wT", [E, D], f16, kind="ExternalInput").ap()
    xp_wT = nc.dram_tensor("xp_wT", [E, R + 2 * N], f16, kind="ExternalInput").ap()
    dtp_wT = nc.dram_tensor("dtp_wT", [R, E], f16, kind="ExternalInput").ap()
    # packed per-partition columns: [conv_w(4) per tile | conv_b | dt_b | D | norm cols]
    convw = nc.dram_tensor("convw", [128, ET * KC], f32, kind="ExternalInput").ap()
    convb = nc.dram_tensor("convb", [128, ET], f32, kind="ExternalInput").ap()
    dtb = nc.dram_tensor("dtb", [128, ET], f32, kind="ExternalInput").ap()
    dvec = nc.dram_tensor("dvec", [128, ET], f32, kind="ExternalInput").ap()
    alog = nc.dram_tensor("alog", [128, ET * N], f32, kind="ExternalInput").ap()
    nw = nc.dram_tensor("nw", [128, DT], f32, kind="ExternalInput").ap()
    nb = nc.dram_tensor("nb", [128, DT], f32, kind="ExternalInput").ap()
    y_part = nc.dram_tensor("y_part", [D, L], f32, kind="ExternalOutput").ap()
    bcd = nc.dram_tensor("bcd", [2 * N, L], f16).ap()  # B/C rows bounce buffer
    zdram = nc.dram_tensor("zdram", [E, L], f16).ap()  # z half spill

    with tile.TileContext(nc) as tc:
        with ExitStack() as ctx:
            P = 128

            def pool(name, bufs):
                return ctx.enter_context(tc.tile_pool(name=name, bufs=bufs))

            pers = pool("pers", 1)
            ps_pool = ctx.enter_context(tc.tile_pool(name="ps", bufs=2, space="PSUM"))
            ps_stat = ctx.enter_context(tc.tile_pool(name="psst", bufs=1, space="PSUM"))
            ps_small = ctx.enter_context(tc.tile_pool(name="pssm", bufs=1, space="PSUM"))
            ps_y = ctx.enter_context(tc.tile_pool(name="psy", bufs=2, space="PSUM"))

            # ---- persistent weight tiles ----
            w_in = [pers.tile([P, 2 * E], f16, tag=f"win{k}") for k in range(DT)]
            for k in range(DT):
                nc.sync.dma_start(w_in[k][:], w_inT[128 * k:128 * (k + 1), :])
            out_w = [pers.tile([P, D], f16, tag=f"ow{i}") for i in range(ET)]
            for i in range(ET):
                nc.sync.dma_start(out_w[i][:], out_wT[128 * i:128 * (i + 1), :])
            xp_w = [pers.tile([P, R + 2 * N], f16, tag=f"xpw{i}") for i in range(ET)]
            for i in range(ET):
                nc.sync.dma_start(xp_w[i][:], xp_wT[128 * i:128 * (i + 1), :])
            dtp_w = pers.tile([R, E], f16, tag="dtpw")
            nc.sync.dma_start(dtp_w[:], dtp_wT[:])
            cw = pers.tile([P, ET * KC], f32, tag="cw")
            nc.sync.dma_start(cw[:], convw[:])
            cb = pers.tile([P, ET], f32, tag="cb")
            nc.sync.dma_start(cb[:], convb[:])
            dtbt = pers.tile([P, ET], f32, tag="dtb")
            nc.sync.dma_start(dtbt[:], dtb[:])
            dvt = pers.tile([P, ET], f32, tag="dv")
            nc.sync.dma_start(dvt[:], dvec[:])
            alg = pers.tile([P, ET * N], f32, tag="alog")
            nc.sync.dma_start(alg[:], alog[:])
            nwt = pers.tile([P, DT], f32, tag="nw")
            nc.sync.dma_start(nwt[:], nw[:])
            nbt = pers.tile([P, DT], f32, tag="nb")
            nc.sync.dma_start(nbt[:], nb[:])

            ones = pers.tile([P, 1], f16, tag="ones")
            nc.vector.memset(ones[:], 1.0)

            # A = -exp(A_log)  (128, ET*N) f32
            At = pers.tile([P, ET * N], f32, tag="A")
            nc.scalar.activation(At[:], alg[:], AF.Exp)
            nc.vector.tensor_scalar_mul(At[:], At[:], -1.0)

            # scan carry state h[:, (i,n)] and conv tails
            carry = [pers.tile([P, N], f16, tag=f"carry{i}") for i in range(ET)]
            xtail = [pers.tile([P, 3], f16, tag=f"xtail{i}") for i in range(ET)]
            for i in range(ET):
                nc.vector.memset(xtail[i][:], 0.0)

            # ---- streaming pools ----
            hst_p = pool("hst", 1)     # hsT chunk tiles (f16)
            sq_p = pool("sq", 1)       # squared tiles (f16)
            srow_p = pool("srow", 1)   # stat rows (1, TL) f32
            rep_p = pool("rep", 1)     # broadcast stat rows (128, TL) f32
            hn_p = pool("hn", 1)       # normalized hs (f16), DT tags
            xpad_p = pool("xpad", 2)   # conv input [3 | TL] f16, cycled
            xc_p = pool("xc", 1)       # conv output f16, ET tags
            z_p = pool("z", 2)         # z spill bounce (small)
            xdbl_p = pool("xdbl", 2)   # (64, TL) f16
            dl_p = pool("dl", 1)       # delta f16, ET tags
            du_p = pool("du", 1)       # delta*u f16, ET tags
            y_p = pool("y", 1)         # y accum f32, ET tags
            bc_p = pool("bc", 3)       # B/C broadcast planes f16
            tr_p = pool("tr", 2)       # scan transients f16
            ov_p = pool("ov", 1)       # out_proj result f32
            tmp_p = pool("tmp", 1)     # misc small

            import itertools
            for rep, c in itertools.product(range(reps), range(NL)):
                lo = c * TL
                # ---------- LayerNorm (over D, layout (D,L)) ----------
                hst = []
                for k in range(DT):
                    t = hst_p.tile([P, TL], f16, tag=f"hst{k}")
                    nc.sync.dma_start(t[:], hsT[128 * k:128 * (k + 1), lo:lo + TL])
                    hst.append(t)
                mu_ps = ps_stat.tile([1, TL], f32, tag="mups")
                sq_ps = ps_stat.tile([1, TL], f32, tag="sqps")
                for k in range(DT):
                    sqt = sq_p.tile([P, TL], f16, tag="sq")
                    nc.scalar.square(sqt[:], hst[k][:])
                    for s in range(NSUB):
                        sl = slice(512 * s, 512 * (s + 1))
                        nc.tensor.matmul(mu_ps[:, sl], ones[:], hst[k][:, sl],
                                         start=(k == 0), stop=(k == DT - 1))
                        nc.tensor.matmul(sq_ps[:, sl], ones[:], sqt[:, sl],
                                         start=(k == 0), stop=(k == DT - 1))
                mu = srow_p.tile([1, TL], f32, tag="mu")
                nc.scalar.activation(mu[:], mu_ps[:], AF.Copy, scale=1.0 / D)
                msq = srow_p.tile([1, TL], f32, tag="msq")
                nc.scalar.activation(msq[:], sq_ps[:], AF.Copy, scale=1.0 / D)
                var = srow_p.tile([1, TL], f32, tag="var")
                nc.vector.tensor_tensor(var[:], mu[:], mu[:], op=op.mult)
                nc.vector.tensor_sub(var[:], msq[:], var[:])
                lnv = srow_p.tile([1, TL], f32, tag="lnv")
                nc.scalar.activation(lnv[:], var[:], AF.Ln, bias=epst[0:1, :])
                rs = srow_p.tile([1, TL], f32, tag="rs")
                nc.scalar.activation(rs[:], lnv[:], AF.Exp, scale=-0.5)
                murep = rep_p.tile([P, TL], f32, tag="murep")
                nc.gpsimd.partition_broadcast(murep[:], mu[:])
                rsrep = rep_p.tile([P, TL], f32, tag="rsrep")
                nc.gpsimd.partition_broadcast(rsrep[:], rs[:])
                hn = []
                for k in range(DT):
                    rsw = tmp_p.tile([P, TL], f32, tag="rsw")
                    nc.vector.tensor_scalar_mul(rsw[:], rsrep[:], nwt[:, k:k + 1])
                    bias2 = tmp_p.tile([P, TL], f32, tag="bias2")
                    nc.vector.tensor_tensor(bias2[:], murep[:], rsw[:], op=op.mult)
                    nc.vector.tensor_scalar(bias2[:], bias2[:], -1.0, nbt[:, k:k + 1],
                                            op0=op.mult, op1=op.add)
                    ht = hn_p.tile([P, TL], f16, tag=f"hn{k}")
                    nc.vector.tensor_tensor(ht[:], hst[k][:], rsw[:], op=op.mult)
                    nc.vector.tensor_add(ht[:], ht[:], bias2[:])
                    hn.append(ht)

                # ---------- in_proj: xz[m, t] ----------
                xpads, zs = [], []
                for m in range(2 * ET):
                    ps = ps_pool.tile([P, TL], f32, tag="mm")
                    for k in range(DT):
                        for s in range(NSUB):
                            sl = slice(512 * s, 512 * (s + 1))
                            nc.tensor.matmul(ps[:, sl],
                                             w_in[k][:, 128 * m:128 * (m + 1)],
                                             hn[k][:, sl],
                                             start=(k == 0), stop=(k == DT - 1))
                    if m < ET:
                        xp = xpad_p.tile([P, TL + 3], f16, tag="xpad")
                        nc.scalar.copy(xp[:, 0:3], xtail[m][:])
                        nc.scalar.copy(xp[:, 3:TL + 3], ps[:])
                        xpads.append(xp)
                    else:
                        zt = z_p.tile([P, TL], f16, tag=f"z{m - ET}")
                        nc.scalar.copy(zt[:], ps[:])
                        zs.append(zt)

                # ---------- causal dwconv + SiLU ----------
                xcs = []
                for i in range(ET):
                    xp = xpads[i]
                    acc = tmp_p.tile([P, TL], f16, tag="cacc")
                    nc.vector.tensor_scalar_mul(acc[:], xp[:, 0:TL], cw[:, KC * i:KC * i + 1])
                    for k in range(1, KC):
                        nc.vector.scalar_tensor_tensor(
                            acc[:], xp[:, k:TL + k], cw[:, KC * i + k:KC * i + k + 1],
                            acc[:], op0=op.mult, op1=op.add)
                    # save tail for next chunk, then silu(acc + conv_b)
                    nc.scalar.copy(xtail[i][:], xp[:, TL:TL + 3])
                    xct = xc_p.tile([P, TL], f16, tag=f"xc{i}")
                    nc.scalar.activation(xct[:], acc[:], AF.Silu, bias=cb[:, i:i + 1])
                    xcs.append(xct)

                # ---------- x_proj ----------
                xdbl = xdbl_p.tile([R + 2 * N, TL], f16, tag="xdbl")
                ps = ps_small.tile([R + 2 * N, TL], f32, tag="xdblps")
                for i in range(ET):
                    for s in range(NSUB):
                        sl = slice(512 * s, 512 * (s + 1))
                        nc.tensor.matmul(ps[:, sl], xp_w[i][:], xcs[i][:, sl],
                                         start=(i == 0), stop=(i == ET - 1))
                nc.scalar.copy(xdbl[:], ps[0:R, :])
                bcs = xdbl_p.tile([2 * N, TL], f16, tag="bcs", name="bcs", bufs=1)
                nc.scalar.copy(bcs[:], ps[R:R + 2 * N, :])
                nc.sync.dma_start(bcd[:, lo:lo + TL], bcs[:])

                # ---------- dt_proj + softplus ----------
                dls, dus, ys = [], [], []
                for i in range(ET):
                    ps = ps_pool.tile([P, TL], f32, tag="mm")
                    for s in range(NSUB):
                        sl = slice(512 * s, 512 * (s + 1))
                        nc.tensor.matmul(ps[:, sl], dtp_w[:, 128 * i:128 * (i + 1)],
                                         xdbl[0:R, sl], start=True, stop=True)
                    esp = tmp_p.tile([P, TL], f32, tag="esp")
                    nc.scalar.activation(esp[:], ps[:], AF.Exp, bias=dtbt[:, i:i + 1])
                    dl = dl_p.tile([P, TL], f16, tag=f"dl{i}")
                    nc.scalar.activation(dl[:], esp[:], AF.Ln, bias=1.0)
                    dls.append(dl)
                    du = du_p.tile([P, TL], f16, tag=f"du{i}")
                    nc.vector.tensor_tensor(du[:], dl[:], xcs[i][:], op=op.mult)
                    dus.append(du)
                    yt = y_p.tile([P, TL], f32, tag=f"y{i}")
                    nc.vector.tensor_scalar_mul(yt[:], xcs[i][:], dvt[:, i:i + 1])
                    ys.append(yt)

                # ---------- selective scan ----------
                for n in range(N):
                    bp = bc_p.tile([P, TL], f16, tag="bp")
                    nc.gpsimd.partition_broadcast(bp[:], xdbl[R + n:R + n + 1, :])
                    cp = bc_p.tile([P, TL], f16, tag="cp")
                    nc.gpsimd.partition_broadcast(cp[:], xdbl[R + N + n:R + N + n + 1, :])
                    for i in range(ET):
                        da = tr_p.tile([P, TL], f16, tag="da")
                        nc.scalar.activation(da[:], dls[i][:], AF.Exp,
                                             scale=At[:, N * i + n:N * i + n + 1])
                        db = tr_p.tile([P, TL], f16, tag="db")
                        nc.vector.tensor_tensor(db[:], dus[i][:], bp[:], op=op.mult)
                        hh = tr_p.tile([P, TL], f16, tag="hh")
                        init = 0.0 if c == 0 else carry[i][:, n:n + 1]
                        nc.vector.tensor_tensor_scan(hh[:], da[:], db[:], init,
                                                     op0=op.mult, op1=op.add)
                        if c < NL - 1:
                            nc.scalar.copy(carry[i][:, n:n + 1], hh[:, TL - 1:TL])
                        gg = tr_p.tile([P, TL], f16, tag="gg")
                        nc.vector.tensor_tensor(gg[:], hh[:], cp[:], op=op.mult)
                        nc.vector.tensor_add(ys[i][:], ys[i][:], gg[:])

                # ---------- gate + out_proj ----------
                ygs = []
                for i in range(ET):
                    zst = tmp_p.tile([P, TL], f16, tag="zs")
                    nc.scalar.activation(zst[:], zs[i][:], AF.Silu)
                    yg = yg_p.tile([P, TL], f16, tag=f"yg{i}")
                    nc.vector.tensor_tensor(yg[:], ys[i][:], zst[:], op=op.mult)
                    ygs.append(yg)
                for m in range(DT):
                    ps = ps_pool.tile([P, TL], f32, tag="mm")
                    for i in range(ET):
                        for s in range(NSUB):
                            sl = slice(512 * s, 512 * (s + 1))
                            nc.tensor.matmul(ps[:, sl],
                                             out_w[i][:, 128 * m:128 * (m + 1)],
                                             ygs[i][:, sl],
                                             start=(i == 0), stop=(i == ET - 1))
                    ov = ov_p.tile([P, TL], f32, tag="ov")
                    nc.scalar.copy(ov[:], ps[:])
                    nc.sync.dma_start(y_part[128 * m:128 * (m + 1), lo:lo + TL], ov[:])

    nc.compile()
    return nc


def _build_stage_b(reps=1):
    import concourse.tile as tile
    from concourse import bacc, mybir
    from concourse.alu_op_type import AluOpType as op
    from contextlib import ExitStack

    dt = mybir.dt
    f32 = dt.float32
    AF = mybir.ActivationFunctionType
    LH = L // 2  # 1024 rows per core

    nc = bacc.Bacc("TRN2", target_bir_lowering=False, debug=False, num_devices=8)
    yf = nc.dram_tensor("yf", [LH, D], f32, kind="ExternalInput").ap()
    yr = nc.dram_tensor("yr", [LH, D], f32, kind="ExternalInput").ap()
    res = nc.dram_tensor("res", [LH, D], f32, kind="ExternalInput").ap()
    nfw = nc.dram_tensor("nfw", [1, D], f32, kind="ExternalInput").ap()
    out = nc.dram_tensor("out", [LH, D], f32, kind="ExternalOutput").ap()

    with tile.TileContext(nc) as tc:
        with ExitStack() as ctx:
            P = 128
            pers = ctx.enter_context(tc.tile_pool(name="pers", bufs=1))
            io_p = ctx.enter_context(tc.tile_pool(name="io", bufs=3))
            tmp_p = ctx.enter_context(tc.tile_pool(name="tmp", bufs=3))

            nfwt = pers.tile([1, D], f32, tag="nfw")
            nc.sync.dma_start(nfwt[:], nfw[:])
            nfr = pers.tile([P, D], f32, tag="nfr")
            nc.gpsimd.partition_broadcast(nfr[:], nfwt[:])

            import itertools
            for rep, t in itertools.product(range(reps), range(LH // P)):
                rows = slice(P * t, P * (t + 1))
                tf = io_p.tile([P, D], f32, tag="tf")
                nc.sync.dma_start(tf[:], yf[rows, :])
                tr = io_p.tile([P, D], f32, tag="tr")
                nc.sync.dma_start(tr[:], yr[rows, :])
                tres = io_p.tile([P, D], f32, tag="tres")
                nc.sync.dma_start(tres[:], res[rows, :])
                s = tmp_p.tile([P, D], f32, tag="s")
                nc.vector.tensor_add(s[:], tf[:], tr[:])
                nc.vector.tensor_add(s[:], s[:], tres[:])
                sq = tmp_p.tile([P, D], f32, tag="sq")
                ssum = tmp_p.tile([P, 1], f32, tag="ssum")
                nc.scalar.activation(sq[:], s[:], AF.Square, accum_out=ssum[:])
                lnm = tmp_p.tile([P, 1], f32, tag="lnm")
                nc.scalar.activation(lnm[:], ssum[:], AF.Ln, bias=epst[:], scale=1.0 / D)
                rinv = tmp_p.tile([P, 1], f32, tag="rinv")
                nc.scalar.activation(rinv[:], lnm[:], AF.Exp, scale=-0.5)
                o = tmp_p.tile([P, D], f32, tag="o")
                nc.vector.scalar_tensor_tensor(o[:], s[:], rinv[:], nfr[:],
                                               op0=op.mult, op1=op.mult)
                nc.sync.dma_start(out[rows, :], o[:])

    nc.compile()
    return nc


class _Runner:
    """Compile a Bass program once into a sharded PJRT callable for 8 cores."""

    def __init__(self, nc, n_cores=8):
        import jax
        import jax.numpy as jnp
        from jax.sharding import Mesh, PartitionSpec
        from jax.experimental.shard_map import shard_map
        from concourse import bass2jax, mybir

        bass2jax.install_neuronx_cc_hook()
        self.n_cores = n_cores
        in_names, out_names, out_avals, zero_outs = [], [], [], []
        partition_name = nc.partition_id_tensor.name if nc.partition_id_tensor else None
        for alloc in nc.m.functions[0].allocations:
            if not isinstance(alloc, mybir.MemoryLocationSet):
                continue
            name = alloc.memorylocations[0].name
            if alloc.kind == "ExternalInput":
                if name != partition_name:
                    in_names.append(name)
            elif alloc.kind == "ExternalOutput":
                shape = tuple(alloc.tensor_shape)
                dtype = mybir.dt.np(alloc.dtype)
                out_names.append(name)
                out_avals.append(jax.core.ShapedArray(shape, dtype))
                zero_outs.append(np.zeros((n_cores * shape[0],) + shape[1:], dtype))
        self.in_names, self.out_names, self.out_avals = in_names, out_names, out_avals
        n_params, n_outs = len(in_names), len(out_names)
        all_names = list(in_names) + list(out_names)
        if partition_name is not None:
            all_names.append(partition_name)

        def _body(*args):
            operands = list(args)
            if partition_name is not None:
                operands.append(bass2jax.partition_id_tensor())
            outs = bass2jax._bass_exec_p.bind(
                *operands,
                out_avals=tuple(out_avals),
                in_names=tuple(all_names),
                out_names=tuple(out_names),
                lowering_input_output_aliases=(),
                sim_require_finite=True,
                sim_require_nnan=True,
                nc=nc,
            )
            return tuple(outs)

        devices = jax.devices()[:n_cores]
        mesh = Mesh(np.asarray(devices), ("core",))
        in_specs = (PartitionSpec("core"),) * (n_params + n_outs)
        out_specs = (PartitionSpec("core"),) * n_outs
        self.fn = jax.jit(
            shard_map(_body, mesh=mesh, in_specs=in_specs,
                      out_specs=out_specs, check_rep=False),
            keep_unused=True)
        self.mesh = mesh
        self._zero_dev = [jax.device_put(z) for z in zero_outs]

    def prep(self, in_maps):
        import jax
        assert len(in_maps) == self.n_cores
        concat = [np.concatenate([np.asarray(m[n]) for m in in_maps], axis=0)
                  for n in self.in_names]
        return [jax.device_put(a) for a in concat] + self._zero_dev

    def run_dev(self, dev_args):
        return self.fn(*dev_args)

    def __call__(self, in_maps):
        import jax
        out_arrs = self.fn(*self.prep(in_maps))
        out_arrs = [np.asarray(a) for a in out_arrs]
        res = []
        for c in range(self.n_cores):
            d = {}
            for i, name in enumerate(self.out_names):
                shape = self.out_avals[i].shape
                d[name] = out_arrs[i].reshape((self.n_cores,) + shape)[c]
            res.append(d)
        return res


def _programs():
    if "a" not in _cache:
        _cache["a"] = _Runner(_build_stage_a())
        _cache["b"] = _Runner(_build_stage_b())
    return _cache["a"], _cache["b"]


def _pack_cols(v, width):
    # (E,)-like flat -> (128, ET*width) per-partition column blocks
    a = np.asarray(v, np.float32).reshape(ET, 128, width)
    return np.ascontiguousarray(a.transpose(1, 0, 2).reshape(128, ET * width))


def kernel(**inputs):
    run_a, run_b = _programs()
    f16 = np.float16
    hs = np.asarray(inputs["hidden_states"], np.float32)

    w_inT = np.ascontiguousarray(np.asarray(inputs["in_proj_w"], np.float32).T).astype(f16)
    out_wT = np.ascontiguousarray(np.asarray(inputs["out_proj_w"], np.float32).T).astype(f16)
    # norm_w/b are per-D; in (D,L) layout D is the partition dim -> column k = rows 128k..128k+127
    nw = np.ascontiguousarray(np.asarray(inputs["norm_w"], np.float32).reshape(DT, 128).T)
    nb = np.ascontiguousarray(np.asarray(inputs["norm_b"], np.float32).reshape(DT, 128).T)

    per_dir = {}
    for d, sfx in ((0, ""), (1, "_b")):
        per_dir[d] = dict(
            xp_wT=np.ascontiguousarray(np.asarray(inputs["x_proj_w" + sfx], np.float32).T).astype(f16),
            dtp_wT=np.ascontiguousarray(np.asarray(inputs["dt_proj_w" + sfx], np.float32).T).astype(f16),
            convw=_pack_cols(inputs["conv_w" + sfx], KC),
            convb=_pack_cols(inputs["conv_b" + sfx], 1),
            dtb=_pack_cols(inputs["dt_proj_b" + sfx], 1),
            alog=_pack_cols(inputs["A_log" if d == 0 else "A_b_log"], N),
            dvec=_pack_cols(inputs["D_fwd" if d == 0 else "D_bwd"], 1),
        )

    in_maps = []
    for c in range(8):
        b, d = c % 4, c // 4
        h = hs[b] if d == 0 else hs[b, ::-1]
        in_maps.append(dict(
            hsT=np.ascontiguousarray(h.T).astype(f16),
            w_inT=w_inT, out_wT=out_wT, nw=nw, nb=nb,
            **per_dir[d],
        ))
    _cache["last_in_maps_a"] = in_maps
    res_a = run_a(in_maps)

    LH = L // 2
    nfw = np.asarray(inputs["normf_w"], np.float32).reshape(1, D)
    in_maps_b = []
    for c in range(8):
        b, half = c % 4, c // 4
        rows = slice(half * LH, (half + 1) * LH)
        yfT = res_a[b]["y_part"].T            # (L, D)
        yrT = res_a[b + 4]["y_part"][:, ::-1].T
        in_maps_b.append(dict(
            yf=np.ascontiguousarray(yfT[rows]),
            yr=np.ascontiguousarray(yrT[rows]),
            res=np.ascontiguousarray(hs[b, rows]),
            nfw=nfw,
        ))
    _cache["last_in_maps_b"] = in_maps_b
    res_b = run_b(in_maps_b)

    out = np.empty((B, L, D), np.float32)
    for c in range(8):
        b, half = c % 4, c // 4
        out[b, half * LH:(half + 1) * LH] = res_b[c]["out"]
    return out
